# revision 10
# baseline (speedup 1.0000x reference)
"""Trainium2 Bass kernel v3 for AttentionBasedTimestamps.

v2 (bf16 + DVE tree reduce) was jointly HBM- and DVE-bound at ~48us:
bf16 wire = 14.7 MB/core (~41us at 358 GB/s) and the 64-way (l,h) combine
needed ~45us of DVE (tensor_reduce runs at 1x mode only).

v3 changes:
  * Wire format fp8-e4m3 (centered, x - 0.5): halves HBM traffic to
    7.34 MB/core (~21us floor). Verified end-to-end rel-err 8.0e-3 vs the
    2e-2 gate (entropy/threshold margins are wide for 64-way averages).
  * The (l,h)-sum moves to the Tensor engine as block-ones matmuls in fp8
    DoubleRow perf mode (0.5 cyc/row): contraction dim = 32 t-rows x 4 lh
    x 2 k-tiles = 8 lh-planes per pass, 8 accumulating passes -> PSUM
    [32,512] f32 per block, ~107ns per matmul, ~6us/core total on PE.
    DVE now only does row stats (~10us), all under the DMA floor.
  * Row layout [t partitions, f free] falls directly out of the matmul --
    no transposes at all.

Sharding: core c handles batch c//2, rows [224*(c%2), 224*(c%2)+224).
Stats round 0 = t rows 0..127, round 1 = rows 128..223 of the core's 224.
"""

import sys

import numpy as np

try:
    import concourse  # noqa: F401
except ImportError:  # pragma: no cover
    sys.path.insert(0, "/opt/trn_rl_repo")

import ml_dtypes

L, B, H, S = 4, 4, 16, 1024
AUDIO_START, AUDIO_END, TEXT_START = 64, 576, 576
FRAME_MS = 40.0
T = S - TEXT_START  # 448
F = AUDIO_END - AUDIO_START  # 512
NS = L * H  # 64
N_CORES = 8
HALVES = 2
RPC = T // HALVES  # 224 rows per core
# Matmul output blocks: DoubleRow matmuls must write PSUM partition 0, and
# regular matmuls may write base 64 (s3d3_mm_valid_dst_partition). Each
# stats round tile [<=128, F] is filled by one 64-row DoubleRow block at
# offset 0 plus one regular-mode block at offset 64:
#   round 0: DR block t 0..63   + regular 64-row block t 64..127
#   round 1: DR block t 128..191 + regular 32-row block t 192..223
NP64 = 16  # DR block: 16 passes x (2 lh x 2 ktiles) = 64 lh-planes
NPR64 = 32  # regular 64-row block: 32 passes x 2 lh
NPR32 = 16  # regular 32-row block: 16 passes x 4 lh
SHIFT = 0.5
THR_ADJ = -0.5 * NS * SHIFT  # -16: A' > 0.5*Amax' + THR_ADJ
HA_THR = -float(NS * SHIFT)  # amax' > -32 <=> has_active

_cache: dict = {}


def _ones_weights(ktiles: int, m: int) -> np.ndarray:
    # lhsT [K=128, (ktiles,) M=m]: out row r sums partitions (128//m)*r ..
    # +128//m of each k-tile.
    w = np.zeros((128, ktiles, m), dtype=ml_dtypes.float8_e4m3)
    for p in range(128):
        w[p, :, p // (128 // m)] = 1.0
    return w if ktiles > 1 else w.reshape(128, m)


def _build_nc(repeat: int = 1):
    import concourse.bacc as bacc
    import concourse.mybir as mybir
    import concourse.tile as tile

    f32 = mybir.dt.float32
    f8 = mybir.dt.float8e4
    i32 = mybir.dt.int32
    Alu = mybir.AluOpType
    Act = mybir.ActivationFunctionType
    X = mybir.AxisListType.X
    DR = mybir.MatmulPerfMode.DoubleRow

    inv_ns = 1.0 / NS
    inv_logf = float(1.0 / np.log(np.float32(F)))

    nc = bacc.Bacc(
        "TRN2", target_bir_lowering=False, debug=False, num_devices=N_CORES
    )
    xd = nc.dram_tensor("xd", [2, 128, NP64, 2, F], f8, kind="ExternalInput")
    xr64 = nc.dram_tensor("xr64", [128, NPR64, F], f8, kind="ExternalInput")
    xr32 = nc.dram_tensor("xr32", [128, NPR32, F], f8, kind="ExternalInput")
    o_i = nc.dram_tensor("o_i", [RPC, 2], i32, kind="ExternalOutput")
    o_f = nc.dram_tensor("o_f", [RPC, 3], f32, kind="ExternalOutput")
    # wd[:, i, :] doubles as the regular 64-row weight (same delta(p//2, m))
    wd_dram = nc.inline_tensor(_ones_weights(2, 64), name="wtsd")
    w32_dram = nc.inline_tensor(_ones_weights(1, 32), name="wts32")

    with tile.TileContext(nc) as tc:
        with (
            tc.tile_pool(name="inp", bufs=4) as inp,
            tc.tile_pool(name="psum", bufs=2, space="PSUM") as psum,
            tc.tile_pool(name="work", bufs=2) as work,
            tc.tile_pool(name="small", bufs=2) as small,
            tc.tile_pool(name="constp", bufs=1) as constp,
            nc.allow_low_precision(reason="fp8 wire format; all sums in f32 PSUM"),
        ):
            iota = constp.tile([128, F], f32, tag="iota")
            nc.gpsimd.iota(
                iota[:],
                pattern=[[1, F]],
                base=0,
                channel_multiplier=0,
                allow_small_or_imprecise_dtypes=True,
            )
            iom = constp.tile([128, F], f32, tag="iom")
            nc.gpsimd.iota(
                iom[:],
                pattern=[[1, F]],
                base=-1000,
                channel_multiplier=0,
                allow_small_or_imprecise_dtypes=True,
            )
            wtsd = constp.tile([128, 2, 64], f8, tag="wtsd")
            nc.sync.dma_start(wtsd[:], wd_dram[:])
            wts32 = constp.tile([128, 32], f8, tag="wts32")
            nc.sync.dma_start(wts32[:], w32_dram[:])

            def row_stats(w, pc, off):
                # Ordered so the scalar-engine Exp runs concurrently with the
                # DVE mask chains (engines execute their streams in program
                # order; the softmax consumers come after the masks so DVE
                # never stalls on the scalar engine).
                A = w[:pc, :]
                amax = small.tile([pc, 1], f32, tag="amax")
                nc.vector.tensor_reduce(amax[:], A, axis=X, op=Alu.max)
                nbias = small.tile([pc, 1], f32, tag="nbias")
                nc.vector.tensor_scalar_mul(nbias[:], amax[:], -inv_ns)
                half = small.tile([pc, 1], f32, tag="half")
                nc.vector.tensor_scalar(
                    half[:], amax[:], 0.5, THR_ADJ, op0=Alu.mult, op1=Alu.add
                )
                ha = small.tile([pc, 1], i32, tag="ha")
                nc.vector.tensor_single_scalar(ha[:], amax[:], HA_THR, Alu.is_gt)
                oi = small.tile([pc, 2], i32, tag="oi")
                of = small.tile([pc, 3], f32, tag="of")

                # kick off softmax exp on the scalar engine early
                e = work.tile([pc, F], f32, tag="e")
                zsum = small.tile([pc, 1], f32, tag="zsum")
                nc.scalar.activation(
                    e[:], A, Act.Exp, bias=nbias[:], scale=inv_ns, accum_out=zsum[:]
                )

                # DVE mask chains overlap the Exp
                t1 = work.tile([pc, F], f32, tag="t1")
                nc.vector.scalar_tensor_tensor(
                    t1[:], A, half[:], iom[:pc, :], Alu.is_gt, Alu.mult
                )
                fi = small.tile([pc, 1], f32, tag="fi")
                nc.vector.tensor_reduce(fi[:], t1[:], axis=X, op=Alu.min)
                t2 = work.tile([pc, F], f32, tag="t2")
                nc.vector.scalar_tensor_tensor(
                    t2[:], A, half[:], iota[:pc, :], Alu.is_gt, Alu.mult
                )
                la = small.tile([pc, 1], f32, tag="la")
                nc.vector.tensor_reduce(la[:], t2[:], axis=X, op=Alu.max)
                t3 = work.tile([pc, F], f32, tag="t3")
                nc.vector.scalar_tensor_tensor(
                    t3[:], A, amax[:], iom[:pc, :], Alu.is_equal, Alu.mult
                )
                pk = small.tile([pc, 1], f32, tag="pk")
                nc.vector.tensor_reduce(pk[:], t3[:], axis=X, op=Alu.min)
                pkt = small.tile([pc, 1], f32, tag="pkt")
                nc.vector.tensor_scalar_add(pkt[:], pk[:], 1000.0)

                stf = small.tile([pc, 1], f32, tag="stf")
                nc.vector.select(stf[:], ha[:], fi[:], pk[:])
                enf = small.tile([pc, 1], f32, tag="enf")
                nc.vector.select(enf[:], ha[:], la[:], pkt[:])

                nc.vector.tensor_scalar_add(oi[:, 0:1], stf[:], 1000.0)
                nc.vector.tensor_copy(oi[:, 1:2], enf[:])
                nc.vector.tensor_scalar(
                    of[:, 0:1], stf[:], FRAME_MS, 1000.0 * FRAME_MS,
                    op0=Alu.mult, op1=Alu.add,
                )
                nc.vector.tensor_scalar_mul(of[:, 1:2], enf[:], FRAME_MS)
                nc.scalar.dma_start(o_i[off : off + pc, :], oi[:])

                # softmax entropy consumers (e/zsum long since ready)
                gg = work.tile([pc, F], f32, tag="gg")
                u64 = small.tile([pc, 1], f32, tag="u64")
                nc.vector.scalar_tensor_tensor(
                    gg[:], A, amax[:], e[:], Alu.subtract, Alu.mult, accum_out=u64[:]
                )
                lnz = small.tile([pc, 1], f32, tag="lnz")
                nc.scalar.activation(lnz[:], zsum[:], Act.Ln, bias=0.0)
                rz = small.tile([pc, 1], f32, tag="rz")
                nc.vector.reciprocal(rz[:], zsum[:])
                s1 = small.tile([pc, 1], f32, tag="s1")
                nc.vector.tensor_single_scalar(s1[:], u64[:], rz[:], Alu.mult)
                sv = small.tile([pc, 1], f32, tag="sv")
                nc.vector.scalar_tensor_tensor(
                    sv[:], s1[:], inv_ns, lnz[:], Alu.mult, Alu.subtract
                )
                nc.vector.tensor_scalar(
                    of[:, 2:3],
                    sv[:],
                    inv_logf,
                    1.0 + F * 1e-9 * inv_logf,
                    op0=Alu.mult,
                    op1=Alu.add,
                )
                nc.scalar.dma_start(o_f[off : off + pc, :], of[:])

            def block_dr(w, blk):
                t = inp.tile([128, NP64, 2, F], f8, tag="ind")
                nc.sync.dma_start(t[:], xd[blk])
                for g in range(NP64):
                    nc.tensor.matmul(
                        w[0:64, :],
                        wtsd[:],
                        t[:, g],
                        start=(g == 0),
                        stop=(g == NP64 - 1),
                        perf_mode=DR,
                    )

            def block_r64(w):
                t = inp.tile([128, NPR64, F], f8, tag="inr")
                nc.sync.dma_start(t[:], xr64[:])
                for g in range(NPR64):
                    nc.tensor.matmul(
                        w[64:128, :],
                        wtsd[:, 0, :],
                        t[:, g],
                        start=(g == 0),
                        stop=(g == NPR64 - 1),
                    )

            def block_r32(w):
                t = inp.tile([128, NPR32, F], f8, tag="inr32")
                nc.sync.dma_start(t[:], xr32[:])
                for g in range(NPR32):
                    nc.tensor.matmul(
                        w[64:96, :],
                        wts32[:],
                        t[:, g],
                        start=(g == 0),
                        stop=(g == NPR32 - 1),
                    )

            for _r in range(repeat):
                # round 0: t rows 0..127
                w = psum.tile([128, F], f32, tag="w")
                block_dr(w, 0)
                block_r64(w)
                row_stats(w, 128, 0)
                # round 1: t rows 128..223
                w = psum.tile([128, F], f32, tag="w")
                block_dr(w, 1)
                block_r32(w)
                row_stats(w, 96, 128)

    nc.compile()
    return nc


def _get_nc():
    if "nc" not in _cache:
        _cache["nc"] = _build_nc()
    return _cache["nc"]


def _prep_in_maps(attn: np.ndarray) -> list[dict]:
    sub = attn[:, :, :, TEXT_START:, AUDIO_START:AUDIO_END]  # [L,B,H,T,F]
    in_maps = []
    for c in range(N_CORES):
        b, hf = divmod(c, HALVES)
        blk = sub[:, b, :, hf * RPC : (hf + 1) * RPC, :]  # [L,H,RPC,F]
        arr = blk.reshape(NS, RPC, F).astype(np.float32) - SHIFT
        q8 = lambda a: a.astype(ml_dtypes.float8_e4m3)
        # DR blocks (t 0..63 and 128..191): xd[blk, p = t64*2 + l2, g, i, f]
        #   = arr[lh = g*4 + i*2 + l2, t, f]
        td = np.stack([arr[:, 0:64], arr[:, 128:192]])  # [blk, lh, t64, f]
        v = td.reshape(2, NP64, 2, 2, 64, F)  # [blk, g, i, l2, t64, f]
        ad = q8(np.ascontiguousarray(v.transpose(0, 4, 3, 1, 2, 5)).reshape(
            2, 128, NP64, 2, F))
        # regular 64-row block (t 64..127): xr64[p = t64*2 + l2, g, f]
        #   = arr[lh = g*2 + l2, t = 64 + t64, f]
        v = arr[:, 64:128].reshape(NPR64, 2, 64, F)  # [g, l2, t64, f]
        a64 = q8(np.ascontiguousarray(v.transpose(2, 1, 0, 3)).reshape(
            128, NPR64, F))
        # regular 32-row block (t 192..223): xr32[p = t32*4 + l4, g, f]
        #   = arr[lh = g*4 + l4, t = 192 + t32, f]
        v = arr[:, 192:224].reshape(NPR32, 4, 32, F)  # [g, l4, t32, f]
        a32 = q8(np.ascontiguousarray(v.transpose(2, 1, 0, 3)).reshape(
            128, NPR32, F))
        in_maps.append({"xd": ad, "xr64": a64, "xr32": a32})
    return in_maps


def _run(in_maps, trace=False, **kw):
    from concourse.bass_utils import run_bass_kernel_spmd

    return run_bass_kernel_spmd(
        _get_nc(), in_maps, list(range(N_CORES)), trace=trace, **kw
    )


def _assemble(results):
    sf = np.empty((B, T), np.int32)
    ef = np.empty((B, T), np.int32)
    sms = np.empty((B, T), np.float32)
    ems = np.empty((B, T), np.float32)
    conf = np.empty((B, T), np.float32)
    for c in range(N_CORES):
        b, hf = divmod(c, HALVES)
        rows = slice(hf * RPC, (hf + 1) * RPC)
        r = results[c]
        sf[b, rows] = r["o_i"][:, 0]
        ef[b, rows] = r["o_i"][:, 1]
        sms[b, rows] = r["o_f"][:, 0]
        ems[b, rows] = r["o_f"][:, 1]
        conf[b, rows] = r["o_f"][:, 2]
    return sf, ef, sms, ems, conf


def _reference_numpy(attn, a0, a1, t0):
    avg = attn.astype(np.float32).mean(axis=(0, 2))
    w = avg[:, t0:, a0:a1]
    nf = w.shape[-1]
    wmax = w.max(axis=-1, keepdims=True)
    peak = w.argmax(axis=-1)
    mask = w > 0.5 * wmax
    has = mask.any(axis=-1)
    first = mask.argmax(axis=-1)
    last = nf - 1 - mask[..., ::-1].argmax(axis=-1)
    startf = np.where(has, first, peak).astype(np.int32)
    endf = np.where(has, last, peak).astype(np.int32)
    m = w.max(axis=-1, keepdims=True)
    ez = np.exp(w - m)
    probs = ez / ez.sum(axis=-1, keepdims=True)
    ent = -(probs * np.log(probs + 1e-9)).sum(axis=-1)
    confv = (1.0 - ent / np.log(np.float32(nf))).astype(np.float32)
    return (
        startf,
        endf,
        (startf * np.float32(FRAME_MS)).astype(np.float32),
        (endf * np.float32(FRAME_MS)).astype(np.float32),
        confv,
    )


def kernel(
    attentions,
    audio_start_idx=AUDIO_START,
    audio_end_idx=AUDIO_END,
    text_start_idx=TEXT_START,
    **_unused,
):
    attn = np.asarray(attentions, dtype=np.float32)
    a0 = int(np.asarray(audio_start_idx))
    a1 = int(np.asarray(audio_end_idx))
    t0 = int(np.asarray(text_start_idx))
    if attn.shape != (L, B, H, S, S) or (a0, a1, t0) != (
        AUDIO_START,
        AUDIO_END,
        TEXT_START,
    ):
        return _reference_numpy(attn, a0, a1, t0)
    in_maps = _prep_in_maps(attn)
    try:
        res = _run(in_maps)
    except Exception:
        try:
            res = _run(in_maps)
        except Exception as ex:  # noqa: BLE001
            sys.stderr.write(f"kernel: device path failed ({ex!r}); CPU fallback\n")
            return _reference_numpy(attn, a0, a1, t0)
    return _assemble(res.results)


# revision 13
# speedup vs baseline: 1.1365x; 1.1365x over previous
"""Trainium2 Bass kernel v3 for AttentionBasedTimestamps.

v2 (bf16 + DVE tree reduce) was jointly HBM- and DVE-bound at ~48us:
bf16 wire = 14.7 MB/core (~41us at 358 GB/s) and the 64-way (l,h) combine
needed ~45us of DVE (tensor_reduce runs at 1x mode only).

v3 changes:
  * Wire format fp8-e4m3 (centered, x - 0.5): halves HBM traffic to
    7.34 MB/core (~21us floor). Verified end-to-end rel-err 8.0e-3 vs the
    2e-2 gate (entropy/threshold margins are wide for 64-way averages).
  * The (l,h)-sum moves to the Tensor engine as block-ones matmuls in fp8
    DoubleRow perf mode (0.5 cyc/row): contraction dim = 32 t-rows x 4 lh
    x 2 k-tiles = 8 lh-planes per pass, 8 accumulating passes -> PSUM
    [32,512] f32 per block, ~107ns per matmul, ~6us/core total on PE.
    DVE now only does row stats (~10us), all under the DMA floor.
  * Row layout [t partitions, f free] falls directly out of the matmul --
    no transposes at all.

Sharding: core c handles batch c//2, rows [224*(c%2), 224*(c%2)+224).
Stats round 0 = t rows 0..127, round 1 = rows 128..223 of the core's 224.
"""

import sys

import numpy as np

try:
    import concourse  # noqa: F401
except ImportError:  # pragma: no cover
    sys.path.insert(0, "/opt/trn_rl_repo")

import ml_dtypes

L, B, H, S = 4, 4, 16, 1024
AUDIO_START, AUDIO_END, TEXT_START = 64, 576, 576
FRAME_MS = 40.0
T = S - TEXT_START  # 448
F = AUDIO_END - AUDIO_START  # 512
NS = L * H  # 64
N_CORES = 8
HALVES = 2
RPC = T // HALVES  # 224 rows per core
# All reduction matmuls run in fp8 DoubleRow mode (2x rate), which the ISA
# only allows at PSUM dst partition 0 (s3d3_mm_valid_dst_partition). Each
# stats round needs [<=128, F], so the lower 64 rows matmul directly into
# the stats tile and the upper block lands in a [64, F] temp at base 0 that
# the scalar engine merges up with an Identity activation (~0.6us, ACT is
# nearly idle):
#   round 0: DR t 0..63 -> W[0:64];   DR t 64..127  -> temp -> W[64:128]
#   round 1: DR t 128..191 -> W[0:64]; DR t 192..223 -> temp -> W[64:96]
NP64 = 16  # 64-row DR block: 16 passes x (2 lh x 2 ktiles) = 64 lh-planes
NP32 = 8  # 32-row DR block: 8 passes x (4 lh x 2 ktiles)
SEG64 = NP64 * 2 * F  # 16384 B/partition per 64-row block
SEG32 = NP32 * 2 * F  # 8192
SEGALL = 3 * SEG64 + SEG32  # 57344 B/partition = whole core input
SHIFT = 0.5
THR_ADJ = -0.5 * NS * SHIFT  # -16: A' > 0.5*Amax' + THR_ADJ
HA_THR = -float(NS * SHIFT)  # amax' > -32 <=> has_active

_cache: dict = {}


def _ones_weights(ktiles: int, m: int) -> np.ndarray:
    # lhsT [K=128, (ktiles,) M=m]: out row r sums partitions (128//m)*r ..
    # +128//m of each k-tile.
    w = np.zeros((128, ktiles, m), dtype=ml_dtypes.float8_e4m3)
    for p in range(128):
        w[p, :, p // (128 // m)] = 1.0
    return w if ktiles > 1 else w.reshape(128, m)


def _build_nc(repeat: int = 1):
    import concourse.bacc as bacc
    import concourse.mybir as mybir
    import concourse.tile as tile

    f32 = mybir.dt.float32
    f8 = mybir.dt.float8e4
    i32 = mybir.dt.int32
    Alu = mybir.AluOpType
    Act = mybir.ActivationFunctionType
    X = mybir.AxisListType.X
    DR = mybir.MatmulPerfMode.DoubleRow

    inv_ns = 1.0 / NS
    inv_logf = float(1.0 / np.log(np.float32(F)))

    nc = bacc.Bacc(
        "TRN2", target_bir_lowering=False, debug=False, num_devices=N_CORES
    )
    # Two DMAs per iteration (one per stats round, ~3.7 MB each): big
    # enough for full DMA efficiency, and compute can start at half-stream.
    xh0 = nc.dram_tensor("xh0", [128, 2 * SEG64], f8, kind="ExternalInput")
    xh1 = nc.dram_tensor("xh1", [128, SEG64 + SEG32], f8, kind="ExternalInput")
    # Single f32 output [sf, ef, s_ms, e_ms, conf]; host casts cols 0-1 to
    # int32 (values are small exact integers). One output DMA per round.
    o5 = nc.dram_tensor("o5", [RPC, 5], f32, kind="ExternalOutput")
    wd_dram = nc.inline_tensor(_ones_weights(2, 64), name="wtsd")
    w32_dram = nc.inline_tensor(_ones_weights(2, 32), name="wts32")

    with tile.TileContext(nc) as tc:
        with (
            tc.tile_pool(name="inp", bufs=2) as inp,
            tc.tile_pool(name="psum", bufs=2, space="PSUM") as psum,
            tc.tile_pool(name="work", bufs=2) as work,
            tc.tile_pool(name="small", bufs=2) as small,
            tc.tile_pool(name="constp", bufs=1) as constp,
            nc.allow_low_precision(reason="fp8 wire format; all sums in f32 PSUM"),
        ):
            iota = constp.tile([128, F], f32, tag="iota")
            nc.gpsimd.iota(
                iota[:],
                pattern=[[1, F]],
                base=0,
                channel_multiplier=0,
                allow_small_or_imprecise_dtypes=True,
            )
            iom = constp.tile([128, F], f32, tag="iom")
            nc.gpsimd.iota(
                iom[:],
                pattern=[[1, F]],
                base=-1000,
                channel_multiplier=0,
                allow_small_or_imprecise_dtypes=True,
            )
            wtsd = constp.tile([128, 2, 64], f8, tag="wtsd")
            nc.sync.dma_start(wtsd[:], wd_dram[:])
            wts32 = constp.tile([128, 2, 32], f8, tag="wts32")
            nc.sync.dma_start(wts32[:], w32_dram[:])

            def stats_mask(w, pc, off):
                # Phase 1 of row stats: threshold masks, first/last/peak,
                # frame outputs, plus the softmax Exp (ACT) and the e*(A-amax)
                # accumulation (DVE) so the PSUM tile is released here.
                # Ordered so the scalar-engine Exp runs concurrently with the
                # DVE mask chains.
                A = w[:pc, :]
                amax = small.tile([pc, 1], f32, tag="amax")
                nc.vector.tensor_reduce(amax[:], A, axis=X, op=Alu.max)
                nbias = small.tile([pc, 1], f32, tag="nbias")
                nc.vector.tensor_scalar_mul(nbias[:], amax[:], -inv_ns)
                half = small.tile([pc, 1], f32, tag="half")
                nc.vector.tensor_scalar(
                    half[:], amax[:], 0.5, THR_ADJ, op0=Alu.mult, op1=Alu.add
                )
                ha = small.tile([pc, 1], i32, tag="ha")
                nc.vector.tensor_single_scalar(ha[:], amax[:], HA_THR, Alu.is_gt)
                ot = small.tile([pc, 5], f32, tag="o5")

                # kick off softmax exp on the scalar engine early
                e = work.tile([pc, F], f32, tag="e")
                zsum = small.tile([pc, 1], f32, tag="zsum")
                nc.scalar.activation(
                    e[:], A, Act.Exp, bias=nbias[:], scale=inv_ns, accum_out=zsum[:]
                )

                # DVE mask chains overlap the Exp
                t1 = work.tile([pc, F], f32, tag="t1")
                nc.vector.scalar_tensor_tensor(
                    t1[:], A, half[:], iom[:pc, :], Alu.is_gt, Alu.mult
                )
                fi = small.tile([pc, 1], f32, tag="fi")
                nc.vector.tensor_reduce(fi[:], t1[:], axis=X, op=Alu.min)
                t2 = work.tile([pc, F], f32, tag="t2")
                nc.vector.scalar_tensor_tensor(
                    t2[:], A, half[:], iota[:pc, :], Alu.is_gt, Alu.mult
                )
                la = small.tile([pc, 1], f32, tag="la")
                nc.vector.tensor_reduce(la[:], t2[:], axis=X, op=Alu.max)
                t3 = work.tile([pc, F], f32, tag="t3")
                nc.vector.scalar_tensor_tensor(
                    t3[:], A, amax[:], iom[:pc, :], Alu.is_equal, Alu.mult
                )
                pk = small.tile([pc, 1], f32, tag="pk")
                nc.vector.tensor_reduce(pk[:], t3[:], axis=X, op=Alu.min)
                pkt = small.tile([pc, 1], f32, tag="pkt")
                nc.vector.tensor_scalar_add(pkt[:], pk[:], 1000.0)

                stf = small.tile([pc, 1], f32, tag="stf")
                nc.vector.select(stf[:], ha[:], fi[:], pk[:])
                enf = small.tile([pc, 1], f32, tag="enf")
                nc.vector.select(enf[:], ha[:], la[:], pkt[:])

                nc.vector.tensor_scalar_add(ot[:, 0:1], stf[:], 1000.0)
                nc.vector.tensor_copy(ot[:, 1:2], enf[:])
                nc.vector.tensor_scalar(
                    ot[:, 2:3], stf[:], FRAME_MS, 1000.0 * FRAME_MS,
                    op0=Alu.mult, op1=Alu.add,
                )
                nc.vector.tensor_scalar_mul(ot[:, 3:4], enf[:], FRAME_MS)

                # last reader of the PSUM tile: u = sum(e * (A - amax))
                gg = work.tile([pc, F], f32, tag="gg")
                u64 = small.tile([pc, 1], f32, tag="u64")
                nc.vector.scalar_tensor_tensor(
                    gg[:], A, amax[:], e[:], Alu.subtract, Alu.mult, accum_out=u64[:]
                )
                return pc, off, zsum, u64, ot

            def stats_entropy(c):
                # Phase 2: entropy -> confidence, then the round's output DMA.
                # Both rounds' Ln calls are issued back-to-back after both
                # Exps, so ACT reloads its function table twice per iteration
                # (each reload is a ~1.3us HBM fetch riding the DMA queues).
                pc, off, zsum, u64, ot = c
                lnz = small.tile([pc, 1], f32, tag="lnz")
                nc.scalar.activation(lnz[:], zsum[:], Act.Ln, bias=0.0)
                rz = small.tile([pc, 1], f32, tag="rz")
                nc.vector.reciprocal(rz[:], zsum[:])
                s1 = small.tile([pc, 1], f32, tag="s1")
                nc.vector.tensor_single_scalar(s1[:], u64[:], rz[:], Alu.mult)
                sv = small.tile([pc, 1], f32, tag="sv")
                nc.vector.scalar_tensor_tensor(
                    sv[:], s1[:], inv_ns, lnz[:], Alu.mult, Alu.subtract
                )
                nc.vector.tensor_scalar(
                    ot[:, 4:5],
                    sv[:],
                    inv_logf,
                    1.0 + F * 1e-9 * inv_logf,
                    op0=Alu.mult,
                    op1=Alu.add,
                )
                nc.scalar.dma_start(o5[off : off + pc, :], ot[:])

            def mm_block(out, v, wts, np_):
                for g in range(np_):
                    nc.tensor.matmul(
                        out,
                        wts[:],
                        v[:, g],
                        start=(g == 0),
                        stop=(g == np_ - 1),
                        perf_mode=DR,
                    )

            for _r in range(repeat):
                th0 = inp.tile([128, 2 * SEG64], f8, tag="in0")
                nc.sync.dma_start(th0[:], xh0[:])
                th1 = inp.tile([128, SEG64 + SEG32], f8, tag="in1")
                nc.sync.dma_start(th1[:], xh1[:])

                def v64(t, b):
                    return t[:, b * SEG64 : (b + 1) * SEG64].rearrange(
                        "p (g i f) -> p g i f", g=NP64, i=2, f=F
                    )

                v32 = th1[:, SEG64:].rearrange(
                    "p (g i f) -> p g i f", g=NP32, i=2, f=F
                )
                # round 0: t rows 0..127 (merge-copy on DVE: ACT switches
                # function tables at ~1.3us a reload, so ACT stays Exp/Ln)
                w = psum.tile([128, F], f32, tag="w")
                up = psum.tile([64, F], f32, tag="wu")
                mm_block(w[0:64, :], v64(th0, 0), wtsd, NP64)
                mm_block(up[:], v64(th0, 1), wtsd, NP64)
                nc.vector.tensor_copy(w[64:128, :], up[:])
                c0 = stats_mask(w, 128, 0)
                # round 1: t rows 128..223
                w = psum.tile([128, F], f32, tag="w")
                up = psum.tile([64, F], f32, tag="wu")
                mm_block(w[0:64, :], v64(th1, 0), wtsd, NP64)
                mm_block(up[0:32, :], v32, wts32, NP32)
                nc.vector.tensor_copy(w[64:96, :], up[0:32, :])
                c1 = stats_mask(w, 96, 128)
                stats_entropy(c0)
                stats_entropy(c1)

    nc.compile()
    return nc


def _get_nc():
    if "nc" not in _cache:
        _cache["nc"] = _build_nc()
    return _cache["nc"]


def _prep_in_maps(attn: np.ndarray) -> list[dict]:
    sub = attn[:, :, :, TEXT_START:, AUDIO_START:AUDIO_END]  # [L,B,H,T,F]
    in_maps = []
    for c in range(N_CORES):
        b, hf = divmod(c, HALVES)
        blk = sub[:, b, :, hf * RPC : (hf + 1) * RPC, :]  # [L,H,RPC,F]
        arr = blk.reshape(NS, RPC, F).astype(np.float32) - SHIFT
        q8 = lambda a: a.astype(ml_dtypes.float8_e4m3)
        # 64-row DR blocks (t 0..191): xd[blk, p = t64*2 + l2, g, i, f]
        #   = arr[lh = g*4 + i*2 + l2, t = blk*64 + t64, f]
        v = arr[:, : 3 * 64, :].reshape(NP64, 2, 2, 3, 64, F)
        ad = q8(np.ascontiguousarray(v.transpose(3, 4, 2, 0, 1, 5)).reshape(
            3, 128, SEG64))
        # 32-row DR block (t 192..223): xd32[p = t32*4 + l4, g, i, f]
        #   = arr[lh = g*8 + i*4 + l4, t = 192 + t32, f]
        v = arr[:, 192:224, :].reshape(NP32, 2, 4, 32, F)  # [g, i, l4, t32, f]
        a32 = q8(np.ascontiguousarray(v.transpose(3, 2, 0, 1, 4)).reshape(
            128, SEG32))
        in_maps.append({
            "xh0": np.concatenate([ad[0], ad[1]], axis=1),
            "xh1": np.concatenate([ad[2], a32], axis=1),
        })
    return in_maps


def _run(in_maps, trace=False, **kw):
    from concourse.bass_utils import run_bass_kernel_spmd

    return run_bass_kernel_spmd(
        _get_nc(), in_maps, list(range(N_CORES)), trace=trace, **kw
    )


def _assemble(results):
    sf = np.empty((B, T), np.int32)
    ef = np.empty((B, T), np.int32)
    sms = np.empty((B, T), np.float32)
    ems = np.empty((B, T), np.float32)
    conf = np.empty((B, T), np.float32)
    for c in range(N_CORES):
        b, hf = divmod(c, HALVES)
        rows = slice(hf * RPC, (hf + 1) * RPC)
        r = results[c]["o5"]
        sf[b, rows] = r[:, 0].astype(np.int32)
        ef[b, rows] = r[:, 1].astype(np.int32)
        sms[b, rows] = r[:, 2]
        ems[b, rows] = r[:, 3]
        conf[b, rows] = r[:, 4]
    return sf, ef, sms, ems, conf


def _reference_numpy(attn, a0, a1, t0):
    avg = attn.astype(np.float32).mean(axis=(0, 2))
    w = avg[:, t0:, a0:a1]
    nf = w.shape[-1]
    wmax = w.max(axis=-1, keepdims=True)
    peak = w.argmax(axis=-1)
    mask = w > 0.5 * wmax
    has = mask.any(axis=-1)
    first = mask.argmax(axis=-1)
    last = nf - 1 - mask[..., ::-1].argmax(axis=-1)
    startf = np.where(has, first, peak).astype(np.int32)
    endf = np.where(has, last, peak).astype(np.int32)
    m = w.max(axis=-1, keepdims=True)
    ez = np.exp(w - m)
    probs = ez / ez.sum(axis=-1, keepdims=True)
    ent = -(probs * np.log(probs + 1e-9)).sum(axis=-1)
    confv = (1.0 - ent / np.log(np.float32(nf))).astype(np.float32)
    return (
        startf,
        endf,
        (startf * np.float32(FRAME_MS)).astype(np.float32),
        (endf * np.float32(FRAME_MS)).astype(np.float32),
        confv,
    )


def kernel(
    attentions,
    audio_start_idx=AUDIO_START,
    audio_end_idx=AUDIO_END,
    text_start_idx=TEXT_START,
    **_unused,
):
    attn = np.asarray(attentions, dtype=np.float32)
    a0 = int(np.asarray(audio_start_idx))
    a1 = int(np.asarray(audio_end_idx))
    t0 = int(np.asarray(text_start_idx))
    if attn.shape != (L, B, H, S, S) or (a0, a1, t0) != (
        AUDIO_START,
        AUDIO_END,
        TEXT_START,
    ):
        return _reference_numpy(attn, a0, a1, t0)
    in_maps = _prep_in_maps(attn)
    try:
        res = _run(in_maps)
    except Exception:
        try:
            res = _run(in_maps)
        except Exception as ex:  # noqa: BLE001
            sys.stderr.write(f"kernel: device path failed ({ex!r}); CPU fallback\n")
            return _reference_numpy(attn, a0, a1, t0)
    return _assemble(res.results)
